# revision 45
# baseline (speedup 1.0000x reference)
"""Trainium2 Bass kernel for CompositionalFC (moe_routing).

Reference computation:
    z[n,b,o] = x[b,i] @ weight[n,i,o] + bias[n,o]
    out[b,o] = relu( sum_n comp_weight[b,n] * z[n,b,o] )

Strategy: data-parallel over batch across 8 NeuronCores (512 rows each,
weight/bias replicated). Matmuls run in fp8e4 DoubleRow perf mode (two
128-deep k-subtiles per instruction, 2x bf16 throughput). To keep fp8
quantization error inside the correctness gate the weights are
mean-centered on host: w~ = w - 0.5, so the combined effective weight
sum_n c[b,n]*w~[n] is zero-mean and the (shared) x-quantization error is
not coherently amplified. The removed mean contributes the exact rank-1
term 0.5*sum_i(x[b,i]) * sum_n(c[b,n]); it is folded — together with the
bias term sum_n c[b,n]*bias[n,o] — into a precomputed bf16 seed tensor
that expert 0's combine uses as its addend, so the accumulators need no
separate initialization pass.

Per expert the PSUM partials are combined into fp32 SBUF accumulators
with acc = z*c' + acc where c' = comp_weight / (SX*SW) undoes the fp8
input scaling. Combines run per 512-wide PSUM bank (k-major matmul order
per ot) so banks recycle quickly, and are split across engines: Vector
does a fused scalar_tensor_tensor straight from PSUM for bt 0,1; for
bt 2,3 the Activation engine drains PSUM with the scale folded in
(zc = c'*z) and GpSimd adds zc into the accumulator (GpSimd cannot read
PSUM, and a single engine cannot keep up with the PE).

Startup is choreographed around two serial resources: DMA triggers issue
one per ~0.65us on the Sync engine, and the DMA engine queues drain FIFO
at ~300 GB/s aggregate — so xT and w[0] stream in interleaved quarters
ahead of everything else, tiny warm-up matmuls keep the PE clock ramping
from the moment the engine preamble ends, and expert 0 runs kp-outer so
its matmuls chase the arriving w[0] quarters.
"""

import sys

for _p in ("/opt/trn_rl_repo",):
    if _p not in sys.path:
        sys.path.insert(0, _p)

from contextlib import ExitStack

import ml_dtypes
import numpy as np

import concourse.bass as bass
import concourse.mybir as mybir
import concourse.tile as tile
from concourse import bacc
from concourse.bass_utils import run_bass_kernel_spmd
from concourse.tile_rust import add_dep_helper

N_CORES = 8
BATCH, IN_DIM, OUT_DIM, N_EXP = 4096, 1024, 1024, 16
BS = BATCH // N_CORES          # 512 batch rows per core
P = 128                        # partitions
BT = BS // P                   # 4 batch tiles per core
KT = IN_DIM // P               # 8 contraction subtiles per expert
KP = KT // 2                   # 4 DoubleRow k-pairs per expert
FD = 512                       # matmul free dim / PSUM bank width (fp32)
NO = OUT_DIM // FD             # 2 output column tiles

SX = 32.0                      # x fp8 pre-scale (|x*SX| <= ~170 < 240)
SW = 256.0                     # centered-weight fp8 pre-scale (|w~*SW| <= 128)

F32 = mybir.dt.float32
BF16 = mybir.dt.bfloat16
FP8 = mybir.dt.float8e4
DBLROW = mybir.MatmulPerfMode.DoubleRow
RELU = mybir.ActivationFunctionType.Relu
COPY = mybir.ActivationFunctionType.Copy
MULT = mybir.AluOpType.mult
ADD = mybir.AluOpType.add


def _build_kernel():
    nc = bacc.Bacc(
        "TRN2",
        target_bir_lowering=False,
        debug=False,
        num_devices=N_CORES,
    )
    xT = nc.declare_dram_parameter("xT", [IN_DIM, BS], FP8, isOutput=False)
    w = nc.declare_dram_parameter("w", [N_EXP, IN_DIM, OUT_DIM], FP8, isOutput=False)
    c = nc.declare_dram_parameter("c", [BS, N_EXP], F32, isOutput=False)
    seed = nc.declare_dram_parameter("seed", [BS, OUT_DIM], BF16, isOutput=False)
    out = nc.declare_dram_parameter("out", [BS, OUT_DIM], F32, isOutput=True)

    with ExitStack() as ctx:
        tc = ctx.enter_context(tile.TileContext(nc))
        const = ctx.enter_context(tc.tile_pool(name="const", bufs=1))
        accp = ctx.enter_context(tc.tile_pool(name="accp", bufs=1))
        wpool = ctx.enter_context(tc.tile_pool(name="wpool", bufs=4))
        zcp = ctx.enter_context(tc.tile_pool(name="zcp", bufs=8))
        psum = ctx.enter_context(tc.tile_pool(name="psum", bufs=8, space="PSUM"))

        # --- HAM warm-up source: no DMA dependency, so the PE can start
        # spinning right after the engine preamble while HBM streams in.
        junk_src = const.tile([P, 2, FD], FP8, tag="junk_src")
        nc.gpsimd.memset(junk_src[:], 0)

        # --- persistent SBUF state -------------------------------------
        xT_sb = const.tile([P, KT, BS], FP8, tag="xT_sb")
        xT_r = xT[:, :].rearrange("(kt p) b -> p kt b", p=P)
        w_sb0 = wpool.tile([P, KT, OUT_DIM], FP8, name="w_sb", tag="w_sb")
        w0_r = w[0, :, :].rearrange("(kp two p) o -> p kp two o", p=P, two=2)
        w0_dmas = []
        # Interleave xT halves with w[0] quarters so expert 0's kp-outer
        # matmul stream chases the arriving data with no dead time.
        nc.gpsimd.dma_start(xT_sb[:, 0:4], xT_r[:, 0:4])
        # c is tiny (32 KB) and gates every combine — its trigger rides
        # second, costing the w[0] stream nothing.
        c_sb = const.tile([P, BT, N_EXP], F32, tag="c_sb")
        nc.gpsimd.dma_start(c_sb[:], c[:, :].rearrange("(bt p) n -> p bt n", p=P))
        w0_dmas.append(nc.gpsimd.dma_start(w_sb0[:, 0:2], w0_r[:, 0]))
        w0_dmas.append(nc.gpsimd.dma_start(w_sb0[:, 2:4], w0_r[:, 1]))
        nc.gpsimd.dma_start(xT_sb[:, 4:8], xT_r[:, 4:8])
        w0_dmas.append(nc.gpsimd.dma_start(w_sb0[:, 4:6], w0_r[:, 2]))
        w0_dmas.append(nc.gpsimd.dma_start(w_sb0[:, 6:8], w0_r[:, 3]))
        # w[1] must beat the seed transfers into the DMA queues: expert 1
        # starts ~17us in, the seed isn't consumed until expert 4.
        w_sb1 = wpool.tile([P, KT, OUT_DIM], FP8, name="w_sb", tag="w_sb")
        nc.gpsimd.dma_start(
            w_sb1[:], w[1, :, :].rearrange("(kt p) o -> p kt o", p=P)
        )
        w_sb2 = wpool.tile([P, KT, OUT_DIM], FP8, name="w_sb", tag="w_sb")
        nc.gpsimd.dma_start(
            w_sb2[:], w[2, :, :].rearrange("(kt p) o -> p kt o", p=P)
        )
        seed_sb = const.tile([P, BT, NO, FD], BF16, tag="seed_sb")
        seed_r = seed[:, :].rearrange(
            "(bt p) (no fd) -> p bt no fd", p=P, fd=FD
        )
        for h in range(BT):
            nc.gpsimd.dma_start(seed_sb[:, h], seed_r[:, h])

        acc = [
            accp.tile([P, NO, FD], F32, name=f"acc_{bt}", tag=f"acc_{bt}")
            for bt in range(BT)
        ]
        # Expert 1 writes its own addend-free accumulator (cheap 2-stream
        # combines while PSUM headroom is scarcest); merged into acc by
        # GpSimd during experts 8..11.
        acc2 = [
            accp.tile([P, NO, FD], F32, name=f"acc2_{bt}", tag=f"acc2_{bt}")
            for bt in range(BT)
        ]

        # --- HAM warm-up: keep the PE clock ramping continuously (an idle
        # gap resets the ramp) while the startup-critical data streams in.
        junk = psum.tile([P, FD], F32, name="junk", tag="zp")
        for _ in range(16):
            nc.tensor.matmul(
                junk[:, 0:256],
                lhsT=junk_src[:, :, 0:P],
                rhs=junk_src[:, :, 0:256],
                start=True,
                stop=True,
                perf_mode=DBLROW,
            )

        # --- main expert loop ------------------------------------------
        out_r = out[:, :].rearrange("(bt p) o -> p bt o", p=P)
        for n in range(N_EXP):
            if n == 0:
                w_sb = w_sb0
            elif n == 1:
                w_sb = w_sb1
            elif n == 2:
                w_sb = w_sb2
            else:
                w_sb = wpool.tile([P, KT, OUT_DIM], FP8, name="w_sb", tag="w_sb")
                nc.sync.dma_start(
                    w_sb[:], w[n, :, :].rearrange("(kt p) o -> p kt o", p=P)
                )
            last = n == N_EXP - 1

            def mm(zp, bt, ot, kp_i):
                nc.tensor.matmul(
                    zp[:],
                    lhsT=xT_sb[:, 2 * kp_i : 2 * kp_i + 2, bt * P : (bt + 1) * P],
                    rhs=w_sb[:, 2 * kp_i : 2 * kp_i + 2, ot * FD : (ot + 1) * FD],
                    start=(kp_i == 0),
                    stop=(kp_i == KP - 1),
                    perf_mode=DBLROW,
                )

            def combine(bt, ot, zp, addend, vec_all=False):
                if bt < 2 or vec_all:
                    # acc = z * c'[:, n] + addend  (c' per-partition scalar)
                    nc.vector.scalar_tensor_tensor(
                        out=acc[bt][:, ot],
                        in0=zp[:],
                        scalar=c_sb[:, bt, n : n + 1],
                        in1=addend,
                        op0=MULT,
                        op1=ADD,
                    )
                else:
                    zc = zcp.tile([P, FD], F32, name="zc", tag="zc")
                    nc.scalar.activation(
                        zc[:], zp[:], COPY, scale=c_sb[:, bt, n : n + 1]
                    )
                    nc.gpsimd.tensor_add(acc[bt][:, ot], zc[:], addend)

            if n == 0:
                # Expert 0: kp-outer (chasing the w[0] quarters as they
                # land) in two bt-waves, so only half of PSUM bursts closed
                # at once. Its combine has no addend (acc = z*c'), which
                # keeps the startup burst cheap: a 2-stream op per half, no
                # GpSimd, no seed dependency — the seed is added later by
                # GpSimd during experts 4..7 where there is engine slack.
                for wave in (0, 1):
                    bts = (2 * wave, 2 * wave + 1)
                    zps = {
                        (bt, ot): psum.tile([P, FD], F32, name="zp", tag="zp")
                        for bt in bts
                        for ot in range(NO)
                    }
                    for kp_i in range(KP):
                        for bt in bts:
                            for ot in range(NO):
                                mm(zps[bt, ot], bt, ot, kp_i)
                    for bt in bts:
                        for ot in range(NO):
                            if bt < 2:
                                nc.vector.tensor_scalar_mul(
                                    acc[bt][:, ot], zps[bt, ot][:],
                                    c_sb[:, bt, 0:1],
                                )
                            else:
                                nc.scalar.activation(
                                    acc[bt][:, ot], zps[bt, ot][:], COPY,
                                    scale=c_sb[:, bt, 0:1],
                                )
                continue

            for bt in range(BT):
                for ot in range(NO):
                    zp = psum.tile([P, FD], F32, name="zp", tag="zp")
                    for kp_i in range(KP):
                        mm(zp, bt, ot, kp_i)
                    if n == 1:
                        # addend-free combine into acc2 (see above)
                        if bt < 2:
                            nc.vector.tensor_scalar_mul(
                                acc2[bt][:, ot], zp[:], c_sb[:, bt, 1:2]
                            )
                        else:
                            nc.scalar.activation(
                                acc2[bt][:, ot], zp[:], COPY,
                                scale=c_sb[:, bt, 1:2],
                            )
                        continue
                    # last expert: all-Vector so the slow GpSimd ops stay off
                    # the kernel tail; relu on Scalar, then store.
                    combine(bt, ot, zp, acc[bt][:, ot], vec_all=last)
                    if n - 8 == bt:
                        nc.gpsimd.tensor_add(
                            acc[bt][:, ot], acc[bt][:, ot], acc2[bt][:, ot]
                        )
                    if n - 4 == bt:
                        # deferred seed add (see expert 0): off the critical
                        # path, on GpSimd's spare capacity.
                        nc.gpsimd.tensor_add(
                            acc[bt][:, ot], acc[bt][:, ot], seed_sb[:, bt, ot]
                        )
                    if last:
                        nc.scalar.activation(acc[bt][:, ot], acc[bt][:, ot], RELU)
                        nc.sync.dma_start(
                            out_r[:, bt, ot * FD : (ot + 1) * FD],
                            acc[bt][:, ot],
                        )

    nc.compile()
    return nc


_NC_CACHE = {}


def _get_nc():
    if "nc" not in _NC_CACHE:
        _NC_CACHE["nc"] = _build_kernel()
    return _NC_CACHE["nc"]


def _fp8(a):
    return np.clip(a, -240.0, 240.0).astype(ml_dtypes.float8_e4m3fn)


def _run(x, comp_weight, weight, bias, trace=False):
    x = np.ascontiguousarray(np.asarray(x, dtype=np.float32))
    comp_weight = np.ascontiguousarray(np.asarray(comp_weight, dtype=np.float32))
    weight = np.asarray(weight, dtype=np.float32)
    bias = np.ascontiguousarray(np.asarray(bias, dtype=np.float32))

    # centered + scaled fp8 weights, shared across cores
    w_q = np.ascontiguousarray(_fp8((weight - 0.5) * SW))
    # seed = c @ bias + exact rank-1 mean correction
    #   t[b] = 0.5 * sum_i x[b,i] * sum_n c[b,n]
    s_full = x.astype(np.float64).sum(axis=1)
    C_full = comp_weight.astype(np.float64).sum(axis=1)
    seed_full = comp_weight.astype(np.float64) @ bias.astype(np.float64)
    seed_full += (0.5 * s_full * C_full)[:, None]
    seed_full = seed_full.astype(np.float32).astype(ml_dtypes.bfloat16)

    in_maps = []
    for r in range(N_CORES):
        sl = slice(r * BS, (r + 1) * BS)
        in_maps.append(
            {
                "xT": np.ascontiguousarray(_fp8(x[sl].T * SX)),
                "w": w_q,
                "c": np.ascontiguousarray(comp_weight[sl] / np.float32(SX * SW)),
                "seed": np.ascontiguousarray(seed_full[sl]),
            }
        )
    res = run_bass_kernel_spmd(
        _get_nc(), in_maps, core_ids=list(range(N_CORES)), trace=trace
    )
    out = np.concatenate([res.results[r]["out"] for r in range(N_CORES)], axis=0)
    return out, res


def kernel(x, comp_weight, weight, bias):
    out, _ = _run(x, comp_weight, weight, bias)
    return out


# revision 46
# speedup vs baseline: 1.0260x; 1.0260x over previous
"""Trainium2 Bass kernel for CompositionalFC (moe_routing).

Reference computation:
    z[n,b,o] = x[b,i] @ weight[n,i,o] + bias[n,o]
    out[b,o] = relu( sum_n comp_weight[b,n] * z[n,b,o] )

Strategy: data-parallel over batch across 8 NeuronCores (512 rows each,
weight/bias replicated). Matmuls run in fp8e4 DoubleRow perf mode (two
128-deep k-subtiles per instruction, 2x bf16 throughput). To keep fp8
quantization error inside the correctness gate the weights are
mean-centered on host: w~ = w - 0.5, so the combined effective weight
sum_n c[b,n]*w~[n] is zero-mean and the (shared) x-quantization error is
not coherently amplified. The removed mean contributes the exact rank-1
term 0.5*sum_i(x[b,i]) * sum_n(c[b,n]); it is folded — together with the
bias term sum_n c[b,n]*bias[n,o] — into a precomputed bf16 seed tensor
that expert 0's combine uses as its addend, so the accumulators need no
separate initialization pass.

Per expert the PSUM partials are combined into fp32 SBUF accumulators
with acc = z*c' + acc where c' = comp_weight / (SX*SW) undoes the fp8
input scaling. Combines run per 512-wide PSUM bank (k-major matmul order
per ot) so banks recycle quickly, and are split across engines: Vector
does a fused scalar_tensor_tensor straight from PSUM for bt 0,1; for
bt 2,3 the Activation engine drains PSUM with the scale folded in
(zc = c'*z) and GpSimd adds zc into the accumulator (GpSimd cannot read
PSUM, and a single engine cannot keep up with the PE).

Startup is choreographed around two serial resources: DMA triggers issue
one per ~0.65us on the Sync engine, and the DMA engine queues drain FIFO
at ~300 GB/s aggregate — so xT and w[0] stream in interleaved quarters
ahead of everything else, tiny warm-up matmuls keep the PE clock ramping
from the moment the engine preamble ends, and expert 0 runs kp-outer so
its matmuls chase the arriving w[0] quarters.
"""

import sys

for _p in ("/opt/trn_rl_repo",):
    if _p not in sys.path:
        sys.path.insert(0, _p)

from contextlib import ExitStack

import ml_dtypes
import numpy as np

import concourse.bass as bass
import concourse.mybir as mybir
import concourse.tile as tile
from concourse import bacc
from concourse.bass_utils import run_bass_kernel_spmd
from concourse.tile_rust import add_dep_helper

N_CORES = 8
BATCH, IN_DIM, OUT_DIM, N_EXP = 4096, 1024, 1024, 16
BS = BATCH // N_CORES          # 512 batch rows per core
P = 128                        # partitions
BT = BS // P                   # 4 batch tiles per core
KT = IN_DIM // P               # 8 contraction subtiles per expert
KP = KT // 2                   # 4 DoubleRow k-pairs per expert
FD = 512                       # matmul free dim / PSUM bank width (fp32)
NO = OUT_DIM // FD             # 2 output column tiles

SX = 32.0                      # x fp8 pre-scale (|x*SX| <= ~170 < 240)
SW = 256.0                     # centered-weight fp8 pre-scale (|w~*SW| <= 128)

F32 = mybir.dt.float32
BF16 = mybir.dt.bfloat16
FP8 = mybir.dt.float8e4
DBLROW = mybir.MatmulPerfMode.DoubleRow
RELU = mybir.ActivationFunctionType.Relu
COPY = mybir.ActivationFunctionType.Copy
MULT = mybir.AluOpType.mult
ADD = mybir.AluOpType.add


def _build_kernel():
    nc = bacc.Bacc(
        "TRN2",
        target_bir_lowering=False,
        debug=False,
        num_devices=N_CORES,
    )
    xT = nc.declare_dram_parameter("xT", [IN_DIM, BS], FP8, isOutput=False)
    w = nc.declare_dram_parameter("w", [N_EXP, IN_DIM, OUT_DIM], FP8, isOutput=False)
    c = nc.declare_dram_parameter("c", [BS, N_EXP], F32, isOutput=False)
    seed = nc.declare_dram_parameter("seed", [BS, OUT_DIM], BF16, isOutput=False)
    out = nc.declare_dram_parameter("out", [BS, OUT_DIM], F32, isOutput=True)

    with ExitStack() as ctx:
        tc = ctx.enter_context(tile.TileContext(nc))
        const = ctx.enter_context(tc.tile_pool(name="const", bufs=1))
        accp = ctx.enter_context(tc.tile_pool(name="accp", bufs=1))
        wpool = ctx.enter_context(tc.tile_pool(name="wpool", bufs=4))
        zcp = ctx.enter_context(tc.tile_pool(name="zcp", bufs=8))
        psum = ctx.enter_context(tc.tile_pool(name="psum", bufs=8, space="PSUM"))

        # --- HAM warm-up source: no DMA dependency, so the PE can start
        # spinning right after the engine preamble while HBM streams in.
        junk_src = const.tile([P, 2, FD], FP8, tag="junk_src")
        nc.gpsimd.memset(junk_src[:], 0)

        # --- persistent SBUF state -------------------------------------
        xT_sb = const.tile([P, KT, BS], FP8, tag="xT_sb")
        xT_r = xT[:, :].rearrange("(kt p) b -> p kt b", p=P)
        w_sb0 = wpool.tile([P, KT, OUT_DIM], FP8, name="w_sb", tag="w_sb")
        w0_r = w[0, :, :].rearrange("(kp two p) o -> p kp two o", p=P, two=2)
        w0_dmas = []
        # Interleave xT halves with w[0] quarters so expert 0's kp-outer
        # matmul stream chases the arriving data with no dead time.
        nc.sync.dma_start(xT_sb[:, 0:4], xT_r[:, 0:4])
        # c is tiny (32 KB) and gates every combine — its trigger rides
        # second, costing the w[0] stream nothing.
        c_sb = const.tile([P, BT, N_EXP], F32, tag="c_sb")
        nc.sync.dma_start(c_sb[:], c[:, :].rearrange("(bt p) n -> p bt n", p=P))
        w0_dmas.append(nc.sync.dma_start(w_sb0[:, 0:2], w0_r[:, 0]))
        w0_dmas.append(nc.sync.dma_start(w_sb0[:, 2:4], w0_r[:, 1]))
        nc.sync.dma_start(xT_sb[:, 4:8], xT_r[:, 4:8])
        w0_dmas.append(nc.sync.dma_start(w_sb0[:, 4:6], w0_r[:, 2]))
        w0_dmas.append(nc.sync.dma_start(w_sb0[:, 6:8], w0_r[:, 3]))
        # w[1] must beat the seed transfers into the DMA queues: expert 1
        # starts ~17us in, the seed isn't consumed until expert 4.
        w_sb1 = wpool.tile([P, KT, OUT_DIM], FP8, name="w_sb", tag="w_sb")
        nc.sync.dma_start(
            w_sb1[:], w[1, :, :].rearrange("(kt p) o -> p kt o", p=P)
        )
        w_sb2 = wpool.tile([P, KT, OUT_DIM], FP8, name="w_sb", tag="w_sb")
        nc.sync.dma_start(
            w_sb2[:], w[2, :, :].rearrange("(kt p) o -> p kt o", p=P)
        )
        seed_sb = const.tile([P, BT, NO, FD], BF16, tag="seed_sb")
        seed_r = seed[:, :].rearrange(
            "(bt p) (no fd) -> p bt no fd", p=P, fd=FD
        )
        for h in range(BT):
            nc.sync.dma_start(seed_sb[:, h], seed_r[:, h])

        acc = [
            accp.tile([P, NO, FD], F32, name=f"acc_{bt}", tag=f"acc_{bt}")
            for bt in range(BT)
        ]
        # Expert 1 writes its own addend-free accumulator (cheap 2-stream
        # combines while PSUM headroom is scarcest); merged into acc by
        # GpSimd during experts 8..11.
        acc2 = [
            accp.tile([P, NO, FD], F32, name=f"acc2_{bt}", tag=f"acc2_{bt}")
            for bt in range(BT)
        ]

        # --- HAM warm-up: keep the PE clock ramping continuously (an idle
        # gap resets the ramp) while the startup-critical data streams in.
        junk = psum.tile([P, FD], F32, name="junk", tag="zp")
        for _ in range(16):
            nc.tensor.matmul(
                junk[:, 0:256],
                lhsT=junk_src[:, :, 0:P],
                rhs=junk_src[:, :, 0:256],
                start=True,
                stop=True,
                perf_mode=DBLROW,
            )

        # --- main expert loop ------------------------------------------
        out_r = out[:, :].rearrange("(bt p) o -> p bt o", p=P)
        for n in range(N_EXP):
            if n == 0:
                w_sb = w_sb0
            elif n == 1:
                w_sb = w_sb1
            elif n == 2:
                w_sb = w_sb2
            else:
                w_sb = wpool.tile([P, KT, OUT_DIM], FP8, name="w_sb", tag="w_sb")
                nc.sync.dma_start(
                    w_sb[:], w[n, :, :].rearrange("(kt p) o -> p kt o", p=P)
                )
            last = n == N_EXP - 1

            def mm(zp, bt, ot, kp_i):
                nc.tensor.matmul(
                    zp[:],
                    lhsT=xT_sb[:, 2 * kp_i : 2 * kp_i + 2, bt * P : (bt + 1) * P],
                    rhs=w_sb[:, 2 * kp_i : 2 * kp_i + 2, ot * FD : (ot + 1) * FD],
                    start=(kp_i == 0),
                    stop=(kp_i == KP - 1),
                    perf_mode=DBLROW,
                )

            def combine(bt, ot, zp, addend, vec_all=False):
                if bt < 2 or vec_all:
                    # acc = z * c'[:, n] + addend  (c' per-partition scalar)
                    nc.vector.scalar_tensor_tensor(
                        out=acc[bt][:, ot],
                        in0=zp[:],
                        scalar=c_sb[:, bt, n : n + 1],
                        in1=addend,
                        op0=MULT,
                        op1=ADD,
                    )
                else:
                    zc = zcp.tile([P, FD], F32, name="zc", tag="zc")
                    nc.scalar.activation(
                        zc[:], zp[:], COPY, scale=c_sb[:, bt, n : n + 1]
                    )
                    nc.gpsimd.tensor_add(acc[bt][:, ot], zc[:], addend)

            if n == 0:
                # Expert 0: kp-outer (chasing the w[0] quarters as they
                # land) in two bt-waves, so only half of PSUM bursts closed
                # at once. Its combine has no addend (acc = z*c'), which
                # keeps the startup burst cheap: a 2-stream op per half, no
                # GpSimd, no seed dependency — the seed is added later by
                # GpSimd during experts 4..7 where there is engine slack.
                for wave in (0, 1):
                    bts = (2 * wave, 2 * wave + 1)
                    zps = {
                        (bt, ot): psum.tile([P, FD], F32, name="zp", tag="zp")
                        for bt in bts
                        for ot in range(NO)
                    }
                    for kp_i in range(KP):
                        for bt in bts:
                            for ot in range(NO):
                                mm(zps[bt, ot], bt, ot, kp_i)
                    for bt in bts:
                        for ot in range(NO):
                            if bt < 2:
                                nc.vector.tensor_scalar_mul(
                                    acc[bt][:, ot], zps[bt, ot][:],
                                    c_sb[:, bt, 0:1],
                                )
                            else:
                                nc.scalar.activation(
                                    acc[bt][:, ot], zps[bt, ot][:], COPY,
                                    scale=c_sb[:, bt, 0:1],
                                )
                continue

            for bt in range(BT):
                for ot in range(NO):
                    zp = psum.tile([P, FD], F32, name="zp", tag="zp")
                    for kp_i in range(KP):
                        mm(zp, bt, ot, kp_i)
                    if n == 1:
                        # addend-free combine into acc2 (see above)
                        if bt < 2:
                            nc.vector.tensor_scalar_mul(
                                acc2[bt][:, ot], zp[:], c_sb[:, bt, 1:2]
                            )
                        else:
                            nc.scalar.activation(
                                acc2[bt][:, ot], zp[:], COPY,
                                scale=c_sb[:, bt, 1:2],
                            )
                        continue
                    # last expert: all-Vector so the slow GpSimd ops stay off
                    # the kernel tail; relu on Scalar, then store.
                    combine(bt, ot, zp, acc[bt][:, ot], vec_all=last)
                    if n - 8 == bt:
                        nc.gpsimd.tensor_add(
                            acc[bt][:, ot], acc[bt][:, ot], acc2[bt][:, ot]
                        )
                    if n - 4 == bt:
                        # deferred seed add (see expert 0): off the critical
                        # path, on GpSimd's spare capacity.
                        nc.gpsimd.tensor_add(
                            acc[bt][:, ot], acc[bt][:, ot], seed_sb[:, bt, ot]
                        )
                    if last:
                        nc.scalar.activation(acc[bt][:, ot], acc[bt][:, ot], RELU)
                        nc.sync.dma_start(
                            out_r[:, bt, ot * FD : (ot + 1) * FD],
                            acc[bt][:, ot],
                        )

    nc.compile()
    return nc


_NC_CACHE = {}


def _get_nc():
    if "nc" not in _NC_CACHE:
        _NC_CACHE["nc"] = _build_kernel()
    return _NC_CACHE["nc"]


def _fp8(a):
    return np.clip(a, -240.0, 240.0).astype(ml_dtypes.float8_e4m3fn)


def _run(x, comp_weight, weight, bias, trace=False):
    x = np.ascontiguousarray(np.asarray(x, dtype=np.float32))
    comp_weight = np.ascontiguousarray(np.asarray(comp_weight, dtype=np.float32))
    weight = np.asarray(weight, dtype=np.float32)
    bias = np.ascontiguousarray(np.asarray(bias, dtype=np.float32))

    # centered + scaled fp8 weights, shared across cores
    w_q = np.ascontiguousarray(_fp8((weight - 0.5) * SW))
    # seed = c @ bias + exact rank-1 mean correction
    #   t[b] = 0.5 * sum_i x[b,i] * sum_n c[b,n]
    s_full = x.astype(np.float64).sum(axis=1)
    C_full = comp_weight.astype(np.float64).sum(axis=1)
    seed_full = comp_weight.astype(np.float64) @ bias.astype(np.float64)
    seed_full += (0.5 * s_full * C_full)[:, None]
    seed_full = seed_full.astype(np.float32).astype(ml_dtypes.bfloat16)

    in_maps = []
    for r in range(N_CORES):
        sl = slice(r * BS, (r + 1) * BS)
        in_maps.append(
            {
                "xT": np.ascontiguousarray(_fp8(x[sl].T * SX)),
                "w": w_q,
                "c": np.ascontiguousarray(comp_weight[sl] / np.float32(SX * SW)),
                "seed": np.ascontiguousarray(seed_full[sl]),
            }
        )
    res = run_bass_kernel_spmd(
        _get_nc(), in_maps, core_ids=list(range(N_CORES)), trace=trace
    )
    out = np.concatenate([res.results[r]["out"] for r in range(N_CORES)], axis=0)
    return out, res


def kernel(x, comp_weight, weight, bias):
    out, _ = _run(x, comp_weight, weight, bias)
    return out
